# revision 1
# baseline (speedup 1.0000x reference)
"""MinibatchDiscrimination kernel for Trainium2 (8 NeuronCores, SPMD).

Problem:  x [256, 1024] f32, T [1024, 128, 32] f32
          M = einsum('ni,iok->nok', x, T)
          norm[a,b,o] = sum_k |M[a,o,k] - M[b,o,k]|
          o_b = exp(-norm).sum(axis=0) - 1            # [256, 128]
          out = concat([x, o_b], axis=1)              # [256, 1152]

Sharding: data-parallel over the out_features axis of T — each of the 8
cores computes the full 256x256 pairwise reduction for 16 output
channels; x is replicated. Host gathers the per-core o_b slices.

Per-core dataflow (pairwise math in bf16 — norms are O(100..4000) and
only reach the output through exp(-norm); bf16 noise cannot move it,
and the diagonal self-term cancels exactly):

  MT[(o,k), a] = Tsh^T @ x^T            PE, [512, 256] in 4 chunks
  ST[o, a]     = sum_k MT               PE (block-ones), kept as bf16

  The DVE ISA has no fused absolute-difference, but |d| = 2*relu(d) - d
  and sum_k d_k = S_a - S_b is rank-1.  So per column b:
     relu tiles:  relu(MT - MT[:,b])    DVE tensor_scalar
                                        (op0=subtract, op1=max, s2=0), bf16 4x
     norm accum:  psum += 2*sum_k relu  PE block-ones(2.0), col-tiled
                  psum += -ST[o, a]     PE selector(-1.0) from ST_bf
     exp:         exp(-psum + (-ST[o,b]))  ACT, bias = per-partition column,
                  accum_out gives sum_a -> o_b[b] + 1
"""

import os as _os_mod
# The axon NTFF profile hook module is absent in this environment; if the
# caller's env has BASS_TRACE set, run_bass_kernel_spmd would crash trying
# to import it.  Force the no-trace path.
_os_mod.environ["BASS_NEVER_TRACE"] = "1"

import numpy as np
import ml_dtypes

import concourse.bass as bass
import concourse.bacc as bacc
import concourse.mybir as mybir
import concourse.tile as tile
from concourse.bass_utils import run_bass_kernel_spmd

BF16 = ml_dtypes.bfloat16

N = 256          # batch
IN_F = 1024      # in features
OUT_F = 128      # out features (total)
K = 32           # kernel dim
NCORES = 8
O = OUT_F // NCORES   # out features per core (16)
NG = N // 4           # pairwise groups of 4 b's (64)


def build_core_program(reps=1, mode="full", n_act=0, n_gps=0):
    nc = bacc.Bacc("TRN2", target_bir_lowering=False)

    xt_d = nc.dram_tensor("xt", [IN_F, N], mybir.dt.bfloat16, kind="ExternalInput")
    tsh_d = nc.dram_tensor("tsh", [IN_F, 4 * 128], mybir.dt.bfloat16, kind="ExternalInput")
    # constant weights: cols 0-31 bones2 (2.0), 32-47 bones1 (1.0), 48-79 negsel (-1.0)
    cw_d = nc.dram_tensor("cw", [128, 80], mybir.dt.bfloat16, kind="ExternalInput")
    # wide constants for m=128 matmuls: 4x band-padded bones2 + negsel4
    cwb_d = nc.dram_tensor("cwb", [128, 656], mybir.dt.bfloat16, kind="ExternalInput")
    ob_d = nc.dram_tensor("ob", [128, NG], mybir.dt.float32, kind="ExternalOutput")
    ob2_d = None
    if mode == "v4":
        ob2_d = nc.dram_tensor("ob2", [16, 128], mybir.dt.float32, kind="ExternalOutput")

    with tile.TileContext(nc) as tc:
        with (
            tc.tile_pool(name="weights", bufs=1) as wpool,
            tc.tile_pool(name="mt", bufs=1) as mtpool,
            tc.tile_pool(name="absd", bufs=int(__import__("os").environ.get("AD_BUFS", "8"))) as adpool,
            tc.tile_pool(name="escratch", bufs=int(__import__("os").environ.get("E_BUFS", "2"))) as epool,
            tc.tile_pool(name="obp", bufs=1) as obpool,
        ):
            import os as _os
            setup_psum = tc.tile_pool(name="psum_mt", bufs=2, space=bass.MemorySpace.PSUM)
            pmt = setup_psum.__enter__()
            psmall_cm = tc.tile_pool(name="psum_s", bufs=1, space=bass.MemorySpace.PSUM)
            psmall = psmall_cm.__enter__()
            # ---- load inputs ----
            cw = wpool.tile([128, 80], mybir.dt.bfloat16)
            nc.sync.dma_start(cw[:], cw_d[:])
            bones2 = cw[:, 0:32]
            bones1 = cw[:, 32:48]
            negsel = cw[:16, 48:80]
            cwb = wpool.tile([128, 656], mybir.dt.bfloat16, tag="cwb")
            nc.sync.dma_start(cwb[:], cwb_d[:])
            bones2band = [cwb[:, 128 * b_l:128 * (b_l + 1)] for b_l in range(4)]
            negsel4 = cwb[:16, 512:640]
            sel16b = cwb[:, 640:656]

            xtl = []
            tshl = []
            for it in range(8):
                xt_t = wpool.tile([128, N], mybir.dt.bfloat16, tag=f"xt{it}")
                nc.sync.dma_start(xt_t[:], xt_d[it * 128:(it + 1) * 128, :])
                xtl.append(xt_t)
                tsh_t = wpool.tile([128, 512], mybir.dt.bfloat16, tag=f"tsh{it}")
                nc.sync.dma_start(tsh_t[:], tsh_d[it * 128:(it + 1) * 128, :])
                tshl.append(tsh_t)

            # ---- MT = Tsh^T @ x^T : [(o,k), a] in 4 chunks of 128 partitions ----
            mt = []      # bf16 working copy
            mtf32 = []   # fp32 upcast of the *bf16-rounded* values (scalar operand)
            for g in range(4):
                pm = pmt.tile([128, N], mybir.dt.float32)
                for it in range(8):
                    nc.tensor.matmul(
                        pm[:],
                        tshl[it][:, g * 128:(g + 1) * 128],
                        xtl[it][:],
                        start=(it == 0),
                        stop=(it == 7),
                    )
                mt_g = mtpool.tile([128, N], mybir.dt.bfloat16, tag=f"mt{g}")
                nc.vector.tensor_copy(mt_g[:], pm[:])
                # fp32 copy MUST come from the bf16 tile so values match exactly
                mt32_g = mtpool.tile([128, N], mybir.dt.float32, tag=f"mt32{g}")
                nc.vector.tensor_copy(mt32_g[:], mt_g[:])
                mt.append(mt_g)
                mtf32.append(mt32_g)
            nmt32 = []
            if n_act > 0:
                for g in range(4):
                    nm_g = mtpool.tile([128, N], mybir.dt.float32, tag=f"nmt32{g}")
                    nc.vector.tensor_scalar(
                        nm_g[:], mt[g][:], -1.0, None, mybir.AluOpType.mult,
                    )
                    nmt32.append(nm_g)

            # ---- ST[o, a] = sum_k MT ----
            st_ps = psmall.tile([16, N], mybir.dt.float32, tag="st_ps")
            for g in range(4):
                nc.tensor.matmul(
                    st_ps[:], bones1[:], mt[g][:], start=(g == 0), stop=(g == 3)
                )
            st_bf = mtpool.tile([16, N], mybir.dt.bfloat16, tag="st_bf")
            nc.vector.tensor_copy(st_bf[:], st_ps[:])

            # ---- bias tile: negSb[32*b_l + o, grp] = -ST_bf[o, 4*grp + b_l] ----
            nsb_ps = psmall.tile([128, NG], mybir.dt.float32, tag="nsb_ps")
            for b_l in range(4):
                nc.tensor.matmul(
                    nsb_ps[32 * b_l:32 * (b_l + 1), :],
                    negsel[:],
                    st_bf[:, b_l::4],
                    start=True,
                    stop=True,
                    tile_position=(0, 32 * b_l),
                )
            negsb = obpool.tile([128, NG], mybir.dt.float32, tag="negsb")
            nc.vector.tensor_copy(negsb[:], nsb_ps[:])

            ob_acc = obpool.tile([128, NG], mybir.dt.float32)
            if mode.startswith("dve_only"):
                nc.vector.memset(ob_acc[:], 0.0)

            # setup-only PSUM pools released; norm pool can take the banks
            psmall_cm.__exit__(None, None, None)
            setup_psum.__exit__(None, None, None)
            pnorm_cm = tc.tile_pool(
                name="psum_norm",
                bufs=int(_os.environ.get("PNORM_BUFS", "7")),
                space=bass.MemorySpace.PSUM,
            )
            pnorm = pnorm_cm.__enter__()
            obt_ps = None
            if mode == "v4":
                obt_cm = tc.tile_pool(name="psum_obt", bufs=1, space=bass.MemorySpace.PSUM)
                obt_pool = obt_cm.__enter__()
                obt_ps = obt_pool.tile([16, 128], mybir.dt.float32, tag="obt")

            # ---- pairwise: groups of 4 b's per PSUM norm tile ----
            import contextlib
            rep_ctx = tc.For_i(0, reps, 1) if reps > 1 else contextlib.nullcontext()
            spread = [0, 8, 4, 12, 2, 10, 6, 14, 1, 9, 5, 13, 3, 11, 7, 15]
            if _os.environ.get("V3_ACT_LAST", "0") == "1":
                spread = [3, 7, 11, 15, 2, 10, 6, 14, 1, 9, 5, 13, 0, 8, 4, 12]
            gps_set = set(spread[:n_gps])
            act_set = set(spread[n_gps:n_gps + n_act])
            n_act_half = int(_os.environ.get("N_ACT_HALF", str(n_act)))
            act_set_half = set(spread[n_gps:n_gps + n_act_half])

            grp_order = list(range(NG))
            if _os.environ.get("INTERLEAVE", "1") == "1" and mode == "v4":
                grp_order = [x for p in zip(range(NG // 2), range(NG // 2, NG)) for x in p]
            with rep_ctx:
              for grp in grp_order:
                  if mode == "v4":
                      half = grp >= NG // 2          # b >= 128: skip a < 128
                      a0 = 128 if half else 0
                      FD = N - a0
                      nt = pnorm.tile([128, FD], mybir.dt.float32,
                                      tag="nt")
                      nc.tensor.matmul(
                          nt[:], negsel4, st_bf[:, a0:], start=True, stop=False,
                      )
                      aset = act_set_half if half else act_set
                      for b_l in range(4):
                          b = 4 * grp + b_l
                          for g in range(4):
                              i = 4 * b_l + g
                              ad = adpool.tile([128, FD], mybir.dt.bfloat16,
                                               tag="ad")
                              if i in aset:
                                  nc.scalar.activation(
                                      ad[:], mt[g][:, a0:],
                                      mybir.ActivationFunctionType.Relu,
                                      bias=nmt32[g][:, b:b + 1],
                                  )
                              else:
                                  nc.vector.tensor_scalar(
                                      ad[:], mt[g][:, a0:], mtf32[g][:, b:b + 1], 0.0,
                                      mybir.AluOpType.subtract, mybir.AluOpType.max,
                                  )
                              nc.tensor.matmul(
                                  nt[:],
                                  bones2band[b_l],
                                  ad[:],
                                  start=False,
                                  stop=(b_l == 3 and g == 3),
                              )
                      e = epool.tile([128, FD], mybir.dt.bfloat16,
                                     tag="e")
                      nc.scalar.activation(
                          e[:], nt[:], mybir.ActivationFunctionType.Exp,
                          scale=-1.0, bias=negsb[:, grp:grp + 1],
                          accum_out=ob_acc[:, grp:grp + 1],
                      )
                      if not half:
                          # transposed contribution: obT[o, a'] += sum_bl E[(bl,o), a']
                          nc.tensor.matmul(
                              obt_ps[:],
                              sel16b[:],
                              e[:, 128:256],
                              start=(grp == 0),
                              stop=(grp == NG // 2 - 1),
                              skip_group_check=True,
                          )
                      continue
                  if mode == "v3":
                      nt = pnorm.tile([128, N], mybir.dt.float32, tag="nt")
                      nc.tensor.matmul(
                          nt[:], negsel4, st_bf[:], start=True, stop=False,
                      )
                      for b_l in range(4):
                          b = 4 * grp + b_l
                          for g in range(4):
                              i = 4 * b_l + g
                              ad = adpool.tile([128, N], mybir.dt.bfloat16, tag="ad")
                              if i in act_set:
                                  nc.scalar.activation(
                                      ad[:], mt[g][:],
                                      mybir.ActivationFunctionType.Relu,
                                      bias=nmt32[g][:, b:b + 1],
                                  )
                              elif i in gps_set:
                                  nc.gpsimd.tensor_scalar(
                                      ad[:], mt[g][:], mtf32[g][:, b:b + 1], 0.0,
                                      mybir.AluOpType.subtract, mybir.AluOpType.max,
                                  )
                              else:
                                  nc.vector.tensor_scalar(
                                      ad[:], mt[g][:], mtf32[g][:, b:b + 1], 0.0,
                                      mybir.AluOpType.subtract, mybir.AluOpType.max,
                                  )
                              nc.tensor.matmul(
                                  nt[:],
                                  bones2band[b_l],
                                  ad[:],
                                  start=False,
                                  stop=(b_l == 3 and g == 3),
                              )
                      if _os.environ.get("EXP_PSUM", "0") == "1":
                          ep = pnorm.tile([128, N], mybir.dt.bfloat16, tag="ep")
                          nc.scalar.activation(
                              ep[:], nt[:], mybir.ActivationFunctionType.Exp,
                              scale=-1.0, bias=negsb[:, grp:grp + 1],
                              accum_out=ob_acc[:, grp:grp + 1],
                          )
                      elif _os.environ.get("EXP_INPLACE", "0") == "1":
                          nc.scalar.activation(
                              nt[:], nt[:], mybir.ActivationFunctionType.Exp,
                              scale=-1.0, bias=negsb[:, grp:grp + 1],
                              accum_out=ob_acc[:, grp:grp + 1],
                          )
                      else:
                          e = epool.tile([128, N], mybir.dt.bfloat16, tag="e")
                          nc.scalar.activation(
                              e[:], nt[:], mybir.ActivationFunctionType.Exp,
                              scale=-1.0, bias=negsb[:, grp:grp + 1],
                              accum_out=ob_acc[:, grp:grp + 1],
                          )
                      continue
                  if mode in ("full_m128", "pe_only_m128"):
                      nt = pnorm.tile([128, N], mybir.dt.float32, tag="nt")
                      nc.tensor.matmul(
                          nt[:], negsel4, st_bf[:], start=True, stop=False,
                      )
                      for b_l in range(4):
                          b = 4 * grp + b_l
                          for g in range(4):
                              ad = None
                              if mode == "full_m128":
                                  ad = adpool.tile([128, N], mybir.dt.bfloat16, tag="ad")
                                  nc.vector.tensor_scalar(
                                      ad[:], mt[g][:], mtf32[g][:, b:b + 1], 0.0,
                                      mybir.AluOpType.subtract, mybir.AluOpType.max,
                                  )
                              nc.tensor.matmul(
                                  nt[:],
                                  bones2band[b_l],
                                  ad[:] if ad is not None else mt[g][:],
                                  start=False,
                                  stop=(b_l == 3 and g == 3),
                              )
                      e = epool.tile([128, N], mybir.dt.bfloat16, tag="e")
                      nc.scalar.activation(
                          e[:], nt[:], mybir.ActivationFunctionType.Exp,
                          scale=-1.0, bias=negsb[:, grp:grp + 1],
                          accum_out=ob_acc[:, grp:grp + 1],
                      )
                      continue
                  use_pe = mode in ("full", "pe_only")
                  use_dve = mode.startswith("dve_only") or mode == "full"
                  nt = None
                  if use_pe:
                      nt = pnorm.tile([128, N], mybir.dt.float32, tag="nt")
                  for b_l in range(4):
                      b = 4 * grp + b_l
                      if use_pe:
                          # -ST[o, a] into this band
                          nc.tensor.matmul(
                              nt[32 * b_l:32 * (b_l + 1), :],
                              negsel[:],
                              st_bf[:],
                              start=True,
                              stop=False,
                              tile_position=(0, 32 * b_l),
                          )
                      for g in range(4):
                          ad = None
                          if use_dve:
                              ad = adpool.tile([128, N], mybir.dt.bfloat16, tag="ad")
                              if mode == "dve_only_subonly":
                                  nc.vector.tensor_scalar(
                                      ad[:], mt[g][:], mtf32[g][:, b:b + 1], None,
                                      mybir.AluOpType.subtract,
                                  )
                              elif mode == "dve_only_bf16s":
                                  nc.vector.tensor_scalar(
                                      ad[:], mt[g][:], mt[g][:, b:b + 1], 0.0,
                                      mybir.AluOpType.subtract, mybir.AluOpType.max,
                                  )
                              else:
                                  nc.vector.tensor_scalar(
                                      ad[:],
                                      mt[g][:],
                                      mtf32[g][:, b:b + 1],
                                      0.0,
                                      mybir.AluOpType.subtract,
                                      mybir.AluOpType.max,
                                  )
                          if use_pe:
                              nc.tensor.matmul(
                                  nt[32 * b_l:32 * (b_l + 1), :],
                                  bones2[:],
                                  ad[:] if (ad is not None and mode == "full") else mt[g][:],
                                  start=False,
                                  stop=(g == 3),
                                  tile_position=(0, 32 * b_l),
                              )
                  if use_pe:
                      e = epool.tile([128, N], mybir.dt.bfloat16, tag="e")
                      nc.scalar.activation(
                          e[:],
                          nt[:],
                          mybir.ActivationFunctionType.Exp,
                          scale=-1.0,
                          bias=negsb[:, grp:grp + 1],
                          accum_out=ob_acc[:, grp:grp + 1],
                      )

            if mode == "v4":
                obt_sb = obpool.tile([16, 128], mybir.dt.float32, tag="obt_sb")
                nc.vector.tensor_copy(obt_sb[:], obt_ps[:])
                nc.sync.dma_start(ob2_d[:], obt_sb[:])
                obt_cm.__exit__(None, None, None)
            pnorm_cm.__exit__(None, None, None)
            ob_final = obpool.tile([128, NG], mybir.dt.float32)
            nc.vector.tensor_scalar_add(ob_final[:], ob_acc[:], -1.0)
            nc.sync.dma_start(ob_d[:], ob_final[:])

    nc.compile()
    return nc


def host_prep_shared(x):
    xt = np.ascontiguousarray(x.T).astype(BF16)
    cw = np.zeros((128, 80), dtype=BF16)
    for p in range(128):
        o = p // 8
        cw[p, o] = 2.0          # bones2
        cw[p, 32 + o] = 1.0     # bones1
    for r in range(16):
        cw[r, 48 + r] = -1.0    # negsel
    cwb = np.zeros((128, 656), dtype=BF16)
    for b_l in range(4):
        for p in range(128):
            cwb[p, 128 * b_l + 32 * b_l + p // 8] = 2.0   # bones2band[b_l]
    for r in range(16):
        for b_l in range(4):
            cwb[r, 512 + 32 * b_l + r] = -1.0             # negsel4
    for p in range(128):
        if p % 32 < 16:
            cwb[p, 640 + (p % 32)] = 1.0                  # sel16b
    return xt, cw, cwb


def pack_tsh(T_core):
    """T_core [IN_F, O, K] -> [IN_F, 512] with col = g*128 + o*8 + k_l, k = 8g + k_l."""
    return np.ascontiguousarray(
        T_core.reshape(IN_F, O, 4, 8).transpose(0, 2, 1, 3).reshape(IN_F, 512)
    ).astype(BF16)


def unscramble(ob_raw):
    """ob_raw [128, NG] f32 -> [N, O]; row = 32*b_l + o, col = grp, n = 4*grp + b_l."""
    a = np.asarray(ob_raw).reshape(4, 32, NG)[:, :O, :]   # [b_l, o, grp]
    return a.transpose(2, 0, 1).reshape(N, O)             # [n, o]


_NC_CACHE = None


def kernel(x, T):
    global _NC_CACHE
    x = np.asarray(x, dtype=np.float32)
    T = np.asarray(T, dtype=np.float32)
    assert x.shape == (N, IN_F) and T.shape == (IN_F, OUT_F, K)

    if _NC_CACHE is None:
        _NC_CACHE = build_core_program(mode="v4", n_act=4)
    nc = _NC_CACHE

    xt, cw, cwb = host_prep_shared(x)
    in_maps = []
    for c in range(NCORES):
        tsh = pack_tsh(T[:, c * O:(c + 1) * O, :])
        in_maps.append({"xt": xt, "tsh": tsh, "cw": cw, "cwb": cwb})

    res = run_bass_kernel_spmd(nc, in_maps, core_ids=list(range(NCORES)))

    cores = []
    for r in res.results:
        ob_c = unscramble(r["ob"])
        # transposed-triangle partial sums: ob[b>=128] += sum_{a<128} E[a, b]
        ob_c[128:, :] += r["ob2"].T
        cores.append(ob_c)
    ob = np.concatenate(cores, axis=1).astype(np.float32)

    out = np.empty((N, IN_F + OUT_F), dtype=np.float32)
    out[:, :IN_F] = x
    out[:, IN_F:] = ob
    return out

